# revision 45
# baseline (speedup 1.0000x reference)
# PRoPE attention Trainium2 kernel (v4).
# Sharding: 8 cores = 2 batches x 4 head-groups (4 heads each).
#
# Design notes (from trace analysis + cost model):
# * ACT exp cost = free_cols*0.833ns + ~242ns fixed -> exp must be wide:
#   one [128,1024] exp per key-tile t (both heads of the pair).
# * Per at-element, exp time == scores+attnv PE time (0.833ns/col each),
#   so attention alone is ACT-bound by the fixed overhead; every attention
#   stretch is overlapped with non-attention PE work (projection chunks,
#   output projection) to keep the PE the pacer.
# * PE p-state: any idle gap drops the PE to half speed for ~3us; the
#   schedule stitches phases (next qb's scores run between the previous
#   qb's last attnv and its Mo matmuls) to avoid gaps.
# * PSUM budget: 8 banks = rotating ring of 3 x [128,1024] slots
#   (scores pss / proj acc+cc / Mo m1+m2 / outproj ys) + 2 x [128,512]
#   attnv accumulators (po).
# * All matmuls bf16 (f32r at <256 cols ran at 4 cyc/col).
# * DVE reciprocal costs ~6.5ns per free element -> denominator runs
#   through the v2-style DRAM double bounce to get a [128,4] shape.
import functools

import numpy as np

B, L, DM = 2, 2048, 1024
H, DH, NG = 16, 64, 16          # heads, head_dim, groups of 4
CAMS, PER_CAM = 8, 256
PX, PY, IW, IH, ROPE_BASE = 16, 16, 256.0, 256.0, 10000.0
HPG = 4                          # heads per group (per core)
HD4 = HPG * DH                   # 256 cols of qkv per core
N_CORES = 8


def _rope_tables():
    """C and S2 tables in (Dh, PER_CAM) layout, tiled to (128, L).
    S2 is the row-pair-swapped S~ table so that
    C*t + S~*swap(t) == C*t + swap(S2*t)."""
    idx = np.arange(PER_CAM)
    u = ((idx % PX) + 0.5) * (IW / PX)
    v = ((idx // PX) + 0.5) * (IH / PY)
    freqs = (np.float32(ROPE_BASE) ** (-(np.arange(NG, dtype=np.float32)) / np.float32(NG)))
    tu = (u[:, None] * freqs[None, :]).astype(np.float32)   # (P, G)
    tv = (v[:, None] * freqs[None, :]).astype(np.float32)
    ca, sa, cb, sb = np.cos(tu), np.sin(tu), np.cos(tv), np.sin(tv)
    Cq = np.zeros((DH, PER_CAM), np.float32)
    Sq = np.zeros((DH, PER_CAM), np.float32)
    for g in range(NG):
        Cq[4 * g + 0] = ca[:, g]; Cq[4 * g + 1] = ca[:, g]
        Cq[4 * g + 2] = cb[:, g]; Cq[4 * g + 3] = cb[:, g]
        # S~ rows: [-sa, sa, -sb, sb]
        Sq[4 * g + 0] = -sa[:, g]; Sq[4 * g + 1] = sa[:, g]
        Sq[4 * g + 2] = -sb[:, g]; Sq[4 * g + 3] = sb[:, g]
    # S2[p] = S~[p^1]
    S2 = np.zeros_like(Sq)
    for p in range(DH):
        S2[p] = Sq[p ^ 1]
    CqL = np.tile(Cq, (1, CAMS))           # (64, 2048)
    S2L = np.tile(S2, (1, CAMS))
    SqL = np.tile(Sq, (1, CAMS))
    csc = np.tile(CqL, (2, 1))             # (128, 2048)
    css2 = np.tile(S2L, (2, 1))
    css = np.tile(SqL, (2, 1))
    return csc, css2, css


def _cam_mats(viewmats, Ks):
    K4 = np.zeros((B, CAMS, 4, 4), np.float32)
    K4[..., :3, :3] = Ks
    K4[..., 3, 3] = 1.0
    P = (K4 @ viewmats).astype(np.float32)
    P_inv = np.linalg.inv(P.astype(np.float64)).astype(np.float32)
    return P, P_inv


@functools.lru_cache(maxsize=1)
def _build_nc():
    import concourse.bass as bass
    import concourse.mybir as mybir
    from concourse.tile import TileContext
    from contextlib import ExitStack

    dt = mybir.dt
    f32 = dt.float32
    bf16 = dt.bfloat16
    ALU = mybir.AluOpType
    ACT = mybir.ActivationFunctionType

    nc = bass.Bass("TRN2", target_bir_lowering=False, debug=False,
                   num_devices=N_CORES)

    xt_d = nc.dram_tensor("xt", [128, 8 * L], bf16, kind="ExternalInput")
    wq_d = nc.dram_tensor("wq", [128, 8 * 256], bf16, kind="ExternalInput")
    wk_d = nc.dram_tensor("wk", [128, 8 * 256], bf16, kind="ExternalInput")
    wv_d = nc.dram_tensor("wv", [128, 8 * 256], bf16, kind="ExternalInput")
    wp_d = nc.dram_tensor("wp", [128, 2 * DM], bf16, kind="ExternalInput")
    csc_d = nc.dram_tensor("csc", [128, L], bf16, kind="ExternalInput")
    css2_d = nc.dram_tensor("css2", [128, L], bf16, kind="ExternalInput")
    css_d = nc.dram_tensor("css", [128, L], bf16, kind="ExternalInput")
    camq_d = nc.dram_tensor("camq", [128, 8 * 128], bf16, kind="ExternalInput")
    camq2_d = nc.dram_tensor("camq2", [128, 8 * 128], bf16, kind="ExternalInput")
    camk_d = nc.dram_tensor("camk", [128, 8 * 128], bf16, kind="ExternalInput")
    camk2_d = nc.dram_tensor("camk2", [128, 8 * 128], bf16, kind="ExternalInput")
    camo_d = nc.dram_tensor("camo", [128, 8 * 128], bf16, kind="ExternalInput")
    camo2_d = nc.dram_tensor("camo2", [128, 8 * 128], bf16, kind="ExternalInput")
    yp_d = nc.dram_tensor("yp", [L, DM], bf16, kind="ExternalOutput")
    bounce2_d = nc.dram_tensor("bounce2", [16, 512], f32, kind="Internal")

    with TileContext(nc) as tc, ExitStack() as ctx:
        # ---- persistent SBUF pools -----------------------------------
        ptab = ctx.enter_context(tc.tile_pool(name="ptab", bufs=3))
        pcam = ctx.enter_context(tc.tile_pool(name="pcam", bufs=6))
        pw = ctx.enter_context(tc.tile_pool(name="pw", bufs=4))
        px = ctx.enter_context(tc.tile_pool(name="px", bufs=1))
        pqk = ctx.enter_context(tc.tile_pool(name="pqk", bufs=4))
        pvt = ctx.enter_context(tc.tile_pool(name="pvt", bufs=2))
        pe12 = ctx.enter_context(tc.tile_pool(name="pe12", bufs=4))
        pat = ctx.enter_context(tc.tile_pool(name="pat", bufs=4))
        popt = ctx.enter_context(tc.tile_pool(name="popt", bufs=8))
        psm = ctx.enter_context(tc.tile_pool(name="psm", bufs=4))
        py = ctx.enter_context(tc.tile_pool(name="py", bufs=3))
        # ---- PSUM: 8 banks -------------------------------------------
        psW = ctx.enter_context(tc.tile_pool(name="psW", bufs=3, space="PSUM"))
        psPo = ctx.enter_context(tc.tile_pool(name="psPo", bufs=2, space="PSUM"))

        # ---- input DMAs ----------------------------------------------
        wq = pw.tile([128, 8 * 256], bf16, tag="w", bufs=4, name="wq")
        wk = pw.tile([128, 8 * 256], bf16, tag="w", bufs=4, name="wk")
        wv = pw.tile([128, 8 * 256], bf16, tag="w", bufs=4, name="wv")
        wp = pw.tile([128, 2 * DM], bf16, tag="w", bufs=4, name="wp")
        xt = px.tile([128, 8 * L], bf16, tag="xt", bufs=1, name="xt")
        xcol = lambda d, lb: 4096 * lb + 512 * d
        # sync queue: xt quarters 0,1; gpsimd queue: weights + xt 2,3;
        # scalar queue: tables + cams (ordered by first use)
        # DMA schedule: ~320 GB/s is fair-shared across all in-flight
        # transfers, so the critical prefix (wk + xt q0 + csc/css2 +
        # camk/camk2 = 2.5MB -> first matmuls at ~12us) is isolated by
        # deferring everything else behind delay ops on the same queues.
        dly = psm.tile([1, 4096], bf16, tag="dly", bufs=1, name="dly")
        csc = ptab.tile([128, L], bf16, tag="tab", bufs=3, name="csc")
        css2 = ptab.tile([128, L], bf16, tag="tab", bufs=3, name="css2")
        css = ptab.tile([128, L], bf16, tag="tab", bufs=3, name="css")
        cam = {}

        def cam_load(nm, dten):
            t = pcam.tile([128, 8 * 128], bf16, tag="cam", bufs=6, name=nm)
            nc.scalar.dma_start(t[:], dten[:, :])
            cam[nm] = t

        nc.gpsimd.dma_start(wk[:], wk_d[:, :])
        nc.sync.dma_start(xt[:, 0:2048], xt_d[:, 0:2048])
        nc.sync.dma_start(xt[:, 2048:4096], xt_d[:, 2048:4096])
        nc.sync.dma_start(xt[:, 4096:8192], xt_d[:, 4096:8192])
        nc.scalar.dma_start(csc[:], csc_d[:, :])
        nc.scalar.dma_start(css2[:], css2_d[:, :])
        cam_load("camk", camk_d)
        cam_load("camk2", camk2_d)
        # wave 2 (gpsimd: after one memset ~3.4us): wv, wq, xt q1
        nc.gpsimd.dma_start(wv[:], wv_d[:, :])
        nc.gpsimd.dma_start(wq[:], wq_d[:, :])
        # wave 2 (scalar: after one copy ~3.7us): q-cams
        cam_load("camq", camq_d)
        cam_load("camq2", camq2_d)
        # wave 3: xt q2/q3, wp, css, o-cams
        nc.gpsimd.dma_start(xt[:, 8192:12288], xt_d[:, 8192:12288])
        nc.gpsimd.dma_start(xt[:, 12288:16384], xt_d[:, 12288:16384])
        nc.gpsimd.dma_start(wp[:], wp_d[:, :])
        cam_load("camo", camo_d)
        cam_load("camo2", camo2_d)
        nc.scalar.dma_start(css[:], css_d[:, :])

        qp = [None, None]
        kp = [None, None]
        vt = [None, None]
        opT = [[None] * 4, [None] * 4]


        for pt in range(2):
            qp[pt] = pqk.tile([128, L], bf16, tag="qk", bufs=4, name=f"qp{pt}")
            kp[pt] = pqk.tile([128, L], bf16, tag="qk", bufs=4, name=f"kp{pt}")
            vt[pt] = pvt.tile([128, 16 * 130], bf16, tag="vt", bufs=2,
                              name=f"vt{pt}")
        # ones columns at (t, 65*hi + 64) for the softmax denominator row
        for pt in range(2):
            v4 = vt[pt][:, :].rearrange("p (t h c) -> p t h c", t=16, h=2, c=65)
            nc.vector.memset(v4[:, :, :, 64:65], 1.0)

        # ---------------- projection chunk ----------------------------
        def proj_chunk(pt, which, lb):
            """One (tensor, lb) unit: 8 proj matmuls + PRoPE transform.
            copies go to ACT for pt0 (idle in prologue) / DVE for pt1."""
            wsl = lambda d: slice(256 * d + 128 * pt, 256 * d + 128 * pt + 128)
            lsl = slice(512 * lb, 512 * lb + 512)
            w8 = {"q": wq, "k": wk, "v": wv}[which]
            slot = psW.tile([128, 1024], f32, tag="w", bufs=3,
                            name=f"pj{pt}{which}{lb}")
            acc = slot[:, 0:512]
            for d in range(8):
                nc.tensor.matmul(acc, w8[:, wsl(d)],
                                 xt[:, xcol(d, lb):xcol(d, lb) + 512],
                                 start=(d == 0), stop=(d == 7))
            e1 = pe12.tile([128, 512], bf16, tag="e", bufs=4, name=f"e1{pt}{which}{lb}")
            e2 = pe12.tile([128, 512], bf16, tag="e", bufs=4, name=f"e2{pt}{which}{lb}")
            nc.vector.tensor_tensor(e1[:], csc[:, lsl], acc, op=ALU.mult)
            nc.vector.tensor_tensor(e2[:], css2[:, lsl], acc, op=ALU.mult)
            if which in ("q", "k"):
                cA = cam["camq"] if which == "q" else cam["camk"]
                cB = cam["camq2"] if which == "q" else cam["camk2"]
                dest = qp[pt] if which == "q" else kp[pt]
                cc = slot[:, 512:1024]
                for ci in range(2):
                    c = 2 * lb + ci
                    csl = slice(256 * ci, 256 * ci + 256)
                    nc.tensor.matmul(cc[:, csl], cA[:, 128 * c:128 * c + 128],
                                     e1[:, csl], start=True, stop=False)
                    nc.tensor.matmul(cc[:, csl], cB[:, 128 * c:128 * c + 128],
                                     e2[:, csl], start=False, stop=True)
                if pt == 0:
                    nc.scalar.copy(dest[:, lsl], cc)
                else:
                    nc.vector.tensor_copy(dest[:, lsl], cc)
            else:
                vo = slot[:, 512:1024]
                for ti in range(4):
                    c = 2 * lb + ti // 2
                    tsl = slice(128 * ti, 128 * ti + 128)
                    nc.tensor.matmul(vo[:, tsl], e1[:, tsl],
                                     cam["camk"][:, 128 * c:128 * c + 128],
                                     start=True, stop=False)
                    nc.tensor.matmul(vo[:, tsl], e2[:, tsl],
                                     cam["camk2"][:, 128 * c:128 * c + 128],
                                     start=False, stop=True)
                # strided copies: vo cols (ti, 64hi+[0:64]) ->
                # vt cols (130*(4lb+ti) + 65hi + [0:64])
                src3 = vo.rearrange("p (t hc) -> p t hc", t=4, hc=128)
                dst3 = vt[pt][:, :].rearrange("p (t x) -> p t x", t=16, x=130)
                for hi in range(2):
                    nc.vector.tensor_copy(
                        dst3[:, 4 * lb:4 * lb + 4, 65 * hi:65 * hi + 64],
                        src3[:, :, 64 * hi:64 * hi + 64])

        # ---------------- attention building blocks --------------------
        at_tiles = {}
        po = [None, None]

        def scores_unit(pt, qb, t):
            qsl = slice(512 * qb, 512 * qb + 512)
            slot = psW.tile([128, 1024], f32, tag="w", bufs=3,
                            name=f"ss{pt}_{qb}_{t}")
            for hi in range(2):
                hsl = slice(64 * hi, 64 * hi + 64)
                nc.tensor.matmul(slot[:, 512 * hi:512 * hi + 512],
                                 kp[pt][hsl, 128 * t:128 * t + 128],
                                 qp[pt][hsl, qsl],
                                 start=True, stop=True,
                                 tile_position=(64 * hi, 0))
            at = pat.tile([128, 1024], bf16, tag="at", bufs=4,
                          name=f"at{pt}_{qb}_{t}")
            nc.scalar.activation(at[:], slot[:], ACT.Exp, scale=0.125)
            at_tiles[t] = at

        def attnv(pt, qb, t):
            for hi in range(2):
                first = (t == 0)
                if first:
                    po[hi] = psPo.tile([128, 512], f32, tag="po", bufs=2,
                                       name=f"po{pt}_{qb}_{hi}")
                nc.tensor.matmul(
                    po[hi][0:65, :],
                    vt[pt][:, 130 * t + 65 * hi:130 * t + 65 * hi + 65],
                    at_tiles[t][:, 512 * hi:512 * hi + 512],
                    start=first, stop=(t == 15))

        def evac_part(pt, qb, on_scalar=False):
            """po -> SBUF + reciprocal of the denominator row (cheap via a
            stream-transpose round trip: reciprocal cost scales with free
            size), broadcast to 64 rows per head via a DRAM bounce.
            The reciprocal chain is emitted first so the bounce DMAs (the
            long-latency part) start as early as possible."""
            ocr = psm.tile([128, 512], bf16, tag="ocr", bufs=2,
                           name=f"ocr{pt}_{qb}")
            rd = psm.tile([128, 512], f32, tag="rd", bufs=2, name=f"rd{pt}_{qb}")
            if not on_scalar:
                # mid-kernel: ocr first — the Mo matmuls (PE filler two
                # units later) must not wait behind the reciprocal chain
                for hi in range(2):
                    nc.vector.tensor_copy(ocr[64 * hi:64 * hi + 64, :],
                                          po[hi][0:64, :])
            for hi in range(2):
                t1 = psm.tile([32, 512], f32, tag="t1", bufs=4,
                              name=f"t1{pt}_{qb}_{hi}")
                nc.vector.transpose(t1[:], po[hi][64:96, :])
                t2 = psm.tile([32, 512], f32, tag="t2", bufs=4,
                              name=f"t2{pt}_{qb}_{hi}")
                t1v = t1[:, :].rearrange("p (j q) -> p j q", j=16, q=32)
                t2v = t2[:, :].rearrange("p (j q) -> p j q", j=16, q=32)
                nc.vector.reciprocal(t2v[:, :, 0:1], t1v[:, :, 0:1])
                rc = psm.tile([32, 512], f32, tag="rc", bufs=4,
                              name=f"rc{pt}_{qb}_{hi}")
                nc.vector.transpose(rc[:], t2[:])
                bi = (pt * 4 + qb) * 2 + hi
                nc.gpsimd.dma_start(bounce2_d[bi:bi + 1, :], rc[0:1, :])
            # second hop emitted after both first hops so the two heads'
            # round trips pipeline instead of serializing on the queue
            for hi in range(2):
                bi = (pt * 4 + qb) * 2 + hi
                nc.gpsimd.dma_start(
                    rd[64 * hi:64 * hi + 64, :],
                    bounce2_d[bi, :][None, :].to_broadcast((64, 512)))
            if on_scalar:
                for hi in range(2):
                    nc.scalar.copy(ocr[64 * hi:64 * hi + 64, :],
                                   po[hi][0:64, :])
            return ocr, rd

        def mo_part(pt, qb, ocr, rd):
            """camo matmuls + rotation + 1/denominator scale -> opT."""
            qsl = slice(512 * qb, 512 * qb + 512)
            slot = psW.tile([128, 1024], f32, tag="w", bufs=3,
                            name=f"mo{pt}_{qb}")
            m1 = slot[:, 0:512]
            m2 = slot[:, 512:1024]
            for ci in range(2):
                c = 2 * qb + ci
                csl = slice(256 * ci, 256 * ci + 256)
                nc.tensor.matmul(m1[:, csl],
                                 cam["camo"][:, 128 * c:128 * c + 128],
                                 ocr[:, csl], start=True, stop=True)
                nc.tensor.matmul(m2[:, csl],
                                 cam["camo2"][:, 128 * c:128 * c + 128],
                                 ocr[:, csl], start=True, stop=True)
            ta = psm.tile([128, 512], f32, tag="mo", bufs=3, name=f"ta{pt}_{qb}")
            tb = psm.tile([128, 512], f32, tag="mo", bufs=3, name=f"tb{pt}_{qb}")
            tc_ = psm.tile([128, 512], f32, tag="mo", bufs=3, name=f"tc{pt}_{qb}")
            nc.vector.tensor_tensor(ta[:], csc[:, qsl], m1, op=ALU.mult)
            nc.vector.tensor_tensor(tb[:], css[:, qsl], m2, op=ALU.mult)
            nc.vector.tensor_tensor(tc_[:], ta[:], tb[:], op=ALU.subtract)
            opT[pt][qb] = popt.tile([128, 512], bf16, tag="opt", bufs=8,
                                    name=f"opT{pt}_{qb}")
            nc.vector.tensor_tensor(opT[pt][qb][:], tc_[:], rd[:], op=ALU.mult)

        def outproj_start(qb, li):
            slot = psW.tile([128, 1024], f32, tag="w", bufs=3,
                            name=f"ys{qb}_{li}")
            tsl = slice(128 * li, 128 * li + 128)
            for nb in range(2):
                nsl = slice(512 * nb, 512 * nb + 512)
                nc.tensor.matmul(slot[:, nsl], opT[0][qb][:, tsl],
                                 wp[:, nsl], start=True, stop=False)
            return slot

        def outproj_stop(qb, li, slot, on_scalar=False):
            lt = 4 * qb + li
            tsl = slice(128 * li, 128 * li + 128)
            for nb in range(2):
                nsl = slice(512 * nb, 512 * nb + 512)
                nc.tensor.matmul(slot[:, nsl], opT[1][qb][:, tsl],
                                 wp[:, DM + nsl.start:DM + nsl.stop],
                                 start=False, stop=True)
            yo = py.tile([128, 1024], bf16, tag="yo", bufs=3,
                         name=f"yo{qb}_{li}")
            if on_scalar:
                nc.scalar.copy(yo[:], slot[:])
            else:
                nc.vector.tensor_copy(yo[:], slot[:])
            nc.sync.dma_start(yp_d[128 * lt:128 * lt + 128, :], yo[:])

        def outproj_unit(qb, li):
            outproj_stop(qb, li, outproj_start(qb, li))

        # ---------------- schedule ------------------------------------
        # prologue: proj(pt0), lb-outer so each xt quarter unlocks 3 chunks
        for lb in range(4):
            for which in ("k", "v", "q"):
                proj_chunk(0, which, lb)

        # fillers keyed by global attention unit index (0..127);
        # attn0 = units 0..63 (pt0), attn1 = 64..127 (pt1)
        fillers = {}

        def add_filler(u, fn):
            fillers.setdefault(u, []).append(fn)

        # attn0: 9 pt1 chunks (k*, v*, q lb0) at t=5,11,15 of qb0..2
        c1 = [("k", lb) for lb in range(4)] + [("v", lb) for lb in range(4)] \
            + [("q", 0)]
        slots0 = [16 * qb + t for qb in range(3) for t in (5, 11, 15)]
        for (which, lb), u in zip(c1, slots0):
            add_filler(u, functools.partial(proj_chunk, 1, which, lb))
        # attn1 qb0: remaining q chunks
        for lb, u in zip((1, 2, 3), (64 + 5, 64 + 9, 64 + 15)):
            add_filler(u, functools.partial(proj_chunk, 1, "q", lb))
        # attn1 qb1..3: outproj of qb-1
        for qb in range(1, 4):
            for li, t in zip(range(4), (3, 9, 15, 15)):
                add_filler(64 + 16 * qb + t,
                           functools.partial(outproj_unit, qb - 1, li))

        pend_mo = []

        def run_attention():
            for u in range(128):
                pt, qb, t = u // 64, (u % 64) // 16, u % 16
                scores_unit(pt, qb, t)
                if t >= 2:
                    attnv(pt, qb, t - 2)
                if t == 1 and pend_mo:
                    mo_part(*pend_mo.pop(0))
                for f in fillers.get(u, ()):
                    f()
                if t == 15:
                    attnv(pt, qb, 14)
                    attnv(pt, qb, 15)
                    ocr, rch = evac_part(pt, qb, on_scalar=(u == 127))
                    pend_mo.append((pt, qb, ocr, rch))

        run_attention()
        # tail: overlap the last Mo chain with outproj(qb3)'s opT[0] half,
        # then keep the PE p-state warm with throwaway matmuls while the
        # denominator bounce completes
        s0 = outproj_start(3, 0)
        s1 = outproj_start(3, 1)
        mo_part(*pend_mo.pop(0))
        s2 = outproj_start(3, 2)
        s3 = outproj_start(3, 3)
        warm = psPo.tile([128, 512], f32, tag="po", bufs=2, name="warm")
        for _ in range(38):
            nc.tensor.matmul(warm[:], wp[:, 0:128], wp[:, 0:512],
                             start=True, stop=True)
        outproj_stop(3, 0, s0, on_scalar=True)
        outproj_stop(3, 1, s1)
        outproj_stop(3, 2, s2, on_scalar=True)
        outproj_stop(3, 3, s3)

    return nc


def _split_multi_waits(nc):
    """This walrus build accepts only one sync-wait per instruction; move
    extras onto standalone InstEventSemaphore ops just before."""
    import concourse.mybir as mybir
    n = 0
    for f in nc.m.functions:
        for bb in f.blocks:
            new_insts = []
            for inst in bb.instructions:
                si = inst.sync_info
                if si is not None and si.on_wait and len(si.on_wait) > 1:
                    waits = list(si.on_wait)
                    for w in waits[:-1]:
                        n += 1
                        new_insts.append(mybir.InstEventSemaphore(
                            name=f"I-splitw-{n}", engine=inst.engine,
                            ins=[], outs=[],
                            sync_info=mybir.SyncInfo(on_wait=[w], on_update=[]),
                        ))
                    inst.sync_info = mybir.SyncInfo(
                        on_wait=[waits[-1]], on_update=list(si.on_update or []))
                new_insts.append(inst)
            bb.instructions = new_insts
    return n


def make_in_maps(x, viewmats, Ks, w_qkv, w_proj):
    import ml_dtypes
    bft = ml_dtypes.bfloat16
    x = np.asarray(x, np.float32)
    viewmats = np.asarray(viewmats, np.float32)
    Ks = np.asarray(Ks, np.float32)
    w_qkv = np.asarray(w_qkv, np.float32)
    w_proj = np.asarray(w_proj, np.float32)

    csc, css2, css = _rope_tables()
    P, P_inv = _cam_mats(viewmats, Ks)
    w3 = w_qkv.reshape(3, H, DH, DM)
    I32 = np.eye(32, dtype=np.float32)
    perm = np.arange(128) ^ 1          # pair swap

    def pack_w(wT):                     # (DM, C) -> (128, 8*C)
        C = wT.shape[1]
        return np.ascontiguousarray(
            wT.reshape(8, 128, C).transpose(1, 0, 2).reshape(128, 8 * C))

    in_maps = []
    for core in range(N_CORES):
        b, hg = divmod(core, HPG)
        heads = slice(4 * hg, 4 * hg + 4)
        xT = np.ascontiguousarray(x[b].T)                        # (DM, L)
        wqT = w3[0, heads].reshape(HD4, DM).T                    # (DM, 256)
        wkT = w3[1, heads].reshape(HD4, DM).T
        wvT = w3[2, heads].reshape(HD4, DM).T
        wpT = w_proj[:, 256 * hg:256 * hg + 256].T               # (256, DM)

        camq = np.stack([np.kron(I32, P_inv[b, c]) for c in range(CAMS)])
        camk = np.stack([np.kron(I32, P[b, c].T) for c in range(CAMS)])
        camo = np.stack([np.kron(I32, P_inv[b, c].T) for c in range(CAMS)])
        camq2 = camq[:, perm, :]        # row pair-swap (lhsT B variant)
        camk2 = camk[:, perm, :]
        camo2 = camo[:, :, perm]        # col pair-swap (output-side variant)

        def pack_cam(cm):               # (8,128,128) -> (128, 8*128)
            return np.ascontiguousarray(
                cm.transpose(1, 0, 2).reshape(128, 8 * 128)).astype(bft)

        xt_p = (xT.reshape(8, 128, 4, 512).transpose(1, 2, 0, 3)
                .reshape(128, 8 * L))  # [p, 4096*lb + 512*d + f]
        in_maps.append({
            "xt": np.ascontiguousarray(xt_p).astype(bft),
            "wq": pack_w(wqT).astype(bft),
            "wk": pack_w(wkT).astype(bft),
            "wv": pack_w(wvT).astype(bft),
            "wp": np.ascontiguousarray(
                wpT.reshape(2, 128, DM).transpose(1, 0, 2).reshape(128, 2 * DM)
            ).astype(bft),
            "csc": csc.astype(bft), "css2": css2.astype(bft),
            "css": css.astype(bft),
            "camq": pack_cam(camq), "camq2": pack_cam(camq2),
            "camk": pack_cam(camk), "camk2": pack_cam(camk2),
            "camo": pack_cam(camo), "camo2": pack_cam(camo2),
        })
    return in_maps


last_results = None


def kernel(x, viewmats, Ks, w_qkv, w_proj):
    from concourse.bass_utils import run_bass_kernel_spmd
    global last_results
    nc = _build_nc()
    if not getattr(nc, "_waits_split", False):
        _split_multi_waits(nc)
        nc._waits_split = True
    in_maps = make_in_maps(x, viewmats, Ks, w_qkv, w_proj)
    res = run_bass_kernel_spmd(nc, in_maps, core_ids=list(range(N_CORES)))
    last_results = res
    outs = res.results
    y = np.zeros((B, L, DM), np.float32)
    for core in range(N_CORES):
        b = core // HPG
        y[b] += outs[core]["yp"].astype(np.float32)
    return y
